# revision 3
# baseline (speedup 1.0000x reference)
"""Trainium2 Bass kernel for LlamaRALAAttention (B=2, S=4096, HID=2048, NH=16, NKV=4, HD=128).

Sharding: 8 cores = DP(batch=2) x TP(kv-head groups=4). Core c handles batch c//4,
kv group c%4 (4 q heads + 1 kv head). o_proj partials summed on host.

Mixed precision (validated vs fp32 reference in sim.py, rel-err ~7.6e-3 vs 2e-2 gate):
  q proj:   fp8e4m3 DoubleRow (x8hi, 64*Wq8), scale folded into rope muls.
  kv proj:  bf16 (as baseline) but kappa(k) kept in fp32 (softmax path is the
            dominant error amplifier; Kk stored f32r, logits computed in fp32).
  phi proj: fp8 DoubleRow 3-term residual (x8hi/x8lo, 64*Wphi hi/lo) ~ bf16 quality
            at half the PE cost.
  o proj:   fp8 DoubleRow 3-term residual: ctx quantized on-chip to an e4m3 hi/lo
            pair (scales 2^-16 / 2^-12), Wo as e4m3 hi/lo pair; psum scaled x1024.
  out:      bf16 partials, host-summed in fp32.

Phases: A) stream 16 chunks of 256 tokens (pre-chunked partition-contiguous DMA
layouts): q/kv/phi projections + rope + kappa, fp32 KkT built via PE transposes.
B) Qg, logits via tiny fp32 matmuls into persistent PSUM, single-global-max
softmax, outer with e-weighted Kk (normalization folded into outer copy).
C) software-pipelined: result matmul + ctx fp8 quantization one chunk ahead of
the o_proj DoubleRow groups; merged 2KB-wide out DMAs on both queues.
"""

import sys

sys.path.insert(0, "/opt/trn_rl_repo")

import numpy as np
import ml_dtypes

import concourse.bass as bass
import concourse.mybir as mybir
import concourse.tile as tile
from concourse import bacc
from concourse.bass_utils import run_bass_kernel_spmd
from concourse.masks import make_identity

P = 128
S = 4096
HID = 2048
HD = 128
NHL = 4            # q heads per core
KO = HID // P      # 16 contraction subtiles
CS = 256           # token chunk size
NCH = S // CS      # 16 chunks
NST = S // P       # 32 s-tiles
ROPE_THETA = 10000.0

F32 = mybir.dt.float32
F32R = mybir.dt.float32r
BF16 = mybir.dt.bfloat16
F8 = mybir.dt.float8e4
DR = mybir.MatmulPerfMode.DoubleRow

BF = ml_dtypes.bfloat16
E4 = ml_dtypes.float8_e4m3

WSCALE = 64.0

_CACHE = {}


def _build():
    nc = bacc.Bacc("TRN2", target_bir_lowering=False, debug=False, num_devices=8)

    x16 = nc.dram_tensor("x16", [NCH, P, KO, CS], BF16, kind="ExternalInput").ap()
    x8h = nc.dram_tensor("x8h", [NCH, P, KO, CS], F8, kind="ExternalInput").ap()
    x8l = nc.dram_tensor("x8l", [NCH, P, KO, CS], F8, kind="ExternalInput").ap()
    cosT = nc.dram_tensor("cosT", [P, S], BF16, kind="ExternalInput").ap()
    sinT = nc.dram_tensor("sinT", [P, S], BF16, kind="ExternalInput").ap()
    cos_sd = nc.dram_tensor("cos_sd", [NCH, P, 2, HD], F32, kind="ExternalInput").ap()
    sin_sd = nc.dram_tensor("sin_sd", [NCH, P, 2, HD], F32, kind="ExternalInput").ap()
    Wq8 = nc.dram_tensor("Wq8", [P, KO, NHL * HD], F8, kind="ExternalInput").ap()
    Wkv = nc.dram_tensor("Wkv", [P, KO, 2 * HD], BF16, kind="ExternalInput").ap()
    Wph = nc.dram_tensor("Wph", [P, KO, NHL * HD], F8, kind="ExternalInput").ap()
    Wpl = nc.dram_tensor("Wpl", [P, KO, NHL * HD], F8, kind="ExternalInput").ap()
    Wo8a = nc.dram_tensor("Wo8a", [P, NHL, HID], F8, kind="ExternalInput").ap()
    Wo8b = nc.dram_tensor("Wo8b", [P, NHL, HID], F8, kind="ExternalInput").ap()
    Wo8r = nc.dram_tensor("Wo8r", [P, NHL, HID], F8, kind="ExternalInput").ap()
    bphi = nc.dram_tensor("bphi", [NHL * HD], F32, kind="ExternalInput").ap()
    RT = nc.dram_tensor("RT", [P, P], BF16, kind="ExternalInput").ap()
    out = nc.dram_tensor("out", [S, HID], BF16, kind="ExternalOutput").ap()

    Wq8_r = Wq8
    Wkv_r = Wkv
    Wph_r = Wph
    Wpl_r = Wpl
    bphi_r = bphi.rearrange("(h p) -> p h", p=P)
    out_r = out.rearrange("(t p) n -> p t n", p=P)

    AX = mybir.AxisListType.X
    MULT = mybir.AluOpType.mult
    ADD = mybir.AluOpType.add
    MAX = mybir.AluOpType.max
    EXP = mybir.ActivationFunctionType.Exp
    IDENT = mybir.ActivationFunctionType.Identity

    from contextlib import ExitStack
    with tile.TileContext(nc) as tc, ExitStack() as es:
        # ---- pools ----
        res = es.enter_context(tc.tile_pool(name="res", bufs=1))
        w16 = es.enter_context(tc.tile_pool(name="w16", bufs=1))   # Wkv
        w8p = es.enter_context(tc.tile_pool(name="w8p", bufs=4))   # Wq8/Wph/Wpl -> Wo ring
        xkv = es.enter_context(tc.tile_pool(name="xkv", bufs=2))   # x16 chunks
        x8p = es.enter_context(tc.tile_pool(name="x8p", bufs=2))   # x8 hi/lo chunks
        tabq = es.enter_context(tc.tile_pool(name="tabq", bufs=2))
        tabk = es.enter_context(tc.tile_pool(name="tabk", bufs=2))
        # (tight SBUF: 224KB/part total)
        stq = es.enter_context(tc.tile_pool(name="stq", bufs=2))   # q-path tmps
        stk = es.enter_context(tc.tile_pool(name="stk", bufs=2))   # k-path tmps
        stb = es.enter_context(tc.tile_pool(name="stb", bufs=2))   # phase-B smalls
        stc = es.enter_context(tc.tile_pool(name="stc", bufs=2))   # phase-C ctx
        sto = es.enter_context(tc.tile_pool(name="sto", bufs=2))   # out tiles
        pq = es.enter_context(tc.tile_pool(name="pq", bufs=2, space="PSUM"))
        pph = es.enter_context(tc.tile_pool(name="pph", bufs=2, space="PSUM"))
        pkv = es.enter_context(tc.tile_pool(name="pkv", bufs=2, space="PSUM"))
        pr = es.enter_context(tc.tile_pool(name="pr", bufs=2, space="PSUM"))

        # ---- weight loads (issue early; Wq8 first for chunk-0 q start) ----
        Wq8_sb = w8p.tile([P, KO, NHL * HD], F8, tag="w8")
        nc.scalar.dma_start(Wq8_sb[:, 0:8, :], Wq8_r[:, 0:8, :])
        Wkv_sb = w16.tile([P, KO, 2 * HD], BF16, tag="w16")
        Wph_sb = w8p.tile([P, KO, NHL * HD], F8, tag="w8")
        Wpl_sb = w8p.tile([P, KO, NHL * HD], F8, tag="w8")
        RT_sb = res.tile([P, P], BF16)
        nc.sync.dma_start(RT_sb[:], RT)
        bphi_sb = res.tile([P, NHL], F32)
        nc.sync.dma_start(bphi_sb[:], bphi_r)

        ident_f32 = res.tile([P, P], F32)
        make_identity(nc, ident_f32[:])
        ones_f32 = res.tile([P, 1], F32)
        nc.vector.memset(ones_f32[:], 1.0)
        onesr_f32 = res.tile([1, P], F32)
        nc.vector.memset(onesr_f32[:], 1.0)
        negr_f32 = res.tile([1, P], F32)
        nc.vector.memset(negr_f32[:], -1.0)
        srow_S = res.tile([1, P], F32)
        nc.vector.memset(srow_S[:], float(S))

        # ---- residents ----
        QkT = res.tile([P, NHL, S], BF16)        # 32KB/part
        phiT = res.tile([P, NHL, S], BF16)       # 32KB/part
        Kk32 = res.tile([P, NST, HD], F32)       # 16KB/part (fp32 kappa(k), [s,d])
        KkT = res.tile([P, S], F32)              # 16KB/part (fp32 kappa(k)^T, [d,s])
        v_sd = res.tile([P, NST, HD], BF16)      # 8KB/part
        qg_parts = res.tile([P, NHL, NCH], F32)
        outer_bf = res.tile([P, NHL, HD], BF16)
        alpha_sd = res.tile([P, NHL, NST], F32)

        inv64 = 1.0 / WSCALE

        # ================= phase A =================
        for c in range(NCH):
            sl = slice(c * CS, (c + 1) * CS)
            xt8h = x8p.tile([P, KO, CS], F8, tag="x8h")
            nc.scalar.dma_start(xt8h[:], x8h[c])
            if c == 0:
                nc.scalar.dma_start(Wq8_sb[:, 8:16, :], Wq8_r[:, 8:16, :])
                nc.scalar.dma_start(Wkv_sb[:, 0:8, :], Wkv_r[:, 0:8, :])
                nc.scalar.dma_start(Wkv_sb[:, 8:16, :], Wkv_r[:, 8:16, :])
            cs_t = tabq.tile([P, CS], BF16, tag="cs")
            nc.sync.dma_start(cs_t[:], cosT[:, sl])
            sn_t = tabq.tile([P, CS], BF16, tag="sn")
            nc.sync.dma_start(sn_t[:], sinT[:, sl])
            xt16 = xkv.tile([P, KO, CS], BF16, tag="x16")
            nc.sync.dma_start(xt16[:], x16[c])
            xt8l = x8p.tile([P, KO, CS], F8, tag="x8l")
            nc.scalar.dma_start(xt8l[:], x8l[c])
            if c == 0:
                nc.sync.dma_start(Wph_sb[:], Wph_r)
                nc.sync.dma_start(Wpl_sb[:], Wpl_r)
            csd = tabk.tile([P, 2, HD], F32, tag="csd")
            nc.sync.dma_start(csd[:], cos_sd[c])
            ssd = tabk.tile([P, 2, HD], F32, tag="ssd")
            nc.sync.dma_start(ssd[:], sin_sd[c])

            # ---- q heads: fp8 DoubleRow proj + rope + kappa ----
            for h in range(NHL):
                psq = pq.tile([P, CS], F32, tag="q")
                for j in range(KO // 2):
                    nc.tensor.matmul(
                        psq[:], Wq8_sb[:, 2 * j:2 * j + 2, h * HD:(h + 1) * HD],
                        xt8h[:, 2 * j:2 * j + 2, :],
                        start=(j == 0), stop=(j == KO // 2 - 1), perf_mode=DR)
                # psq = 64*q ; qs = q*sin, qro = q*cos (+ rotation)
                qs = stq.tile([P, CS], BF16, tag="qs")
                nc.vector.scalar_tensor_tensor(qs[:], psq[:], inv64, sn_t[:], MULT, MULT)
                psr = pr.tile([P, CS], F32, tag="r")
                nc.tensor.matmul(psr[:], RT_sb[:], qs[:], start=True, stop=True)
                qro = stq.tile([P, CS], F32, tag="qro")
                nc.vector.scalar_tensor_tensor(qro[:], psq[:], inv64, cs_t[:], MULT, MULT)
                nc.vector.tensor_add(qro[:], qro[:], psr[:])
                # kappa(x) = max(x,0) + min(exp(x),1)
                eq = stq.tile([P, CS], F32, tag="eq")
                nc.scalar.activation(eq[:], qro[:], EXP)
                nc.gpsimd.tensor_scalar_min(eq[:], eq[:], 1.0)
                nc.vector.scalar_tensor_tensor(
                    QkT[:, h, sl], qro[:], 0.0, eq[:], MAX, ADD)
                nc.vector.tensor_reduce(
                    qg_parts[:, h, c:c + 1], QkT[:, h, sl], AX, ADD)

            # ---- k/v (bf16 proj, fp32 rope+kappa for k) ----
            for st in range(CS // P):
                stg = c * (CS // P) + st
                pskv = pkv.tile([P, 2 * HD], F32, tag="kv")
                for ko in range(KO):
                    nc.tensor.matmul(
                        pskv[:], xt16[:, ko, st * P:(st + 1) * P], Wkv_sb[:, ko, :],
                        start=(ko == 0), stop=(ko == KO - 1))
                nc.scalar.copy(v_sd[:, stg, :], pskv[:, HD:])
                k_ps = pskv[:, :HD]
                kr = stk.tile([P, HD], F32, tag="kr")
                nc.vector.tensor_mul(kr[:], k_ps, csd[:, st, :])
                ta = stk.tile([P, 64], F32, tag="ta")
                nc.vector.tensor_mul(ta[:], k_ps[:, 64:], ssd[:, st, :64])
                nc.vector.tensor_sub(kr[:, :64], kr[:, :64], ta[:])
                tb = stk.tile([P, 64], F32, tag="tb")
                nc.vector.tensor_mul(tb[:], k_ps[:, :64], ssd[:, st, 64:])
                nc.vector.tensor_add(kr[:, 64:], kr[:, 64:], tb[:])
                ek = stk.tile([P, HD], F32, tag="ek")
                nc.scalar.activation(ek[:], kr[:], EXP)
                nc.gpsimd.tensor_scalar_min(ek[:], ek[:], 1.0)
                nc.vector.scalar_tensor_tensor(
                    Kk32[:, stg, :], kr[:], 0.0, ek[:], MAX, ADD)
                pkt = pr.tile([P, P], F32, tag="r", name=f"pkt{stg}")
                nc.tensor.transpose(pkt[:], Kk32[:, stg, :], ident_f32[:])
                nc.scalar.copy(KkT[:, stg * P:(stg + 1) * P], pkt[:])

            # ---- phi: 3-term fp8 residual DoubleRow ----
            for h in range(NHL):
                psp = pph.tile([P, CS], F32, tag="ph")
                terms = [(Wph_sb, xt8h), (Wph_sb, xt8l), (Wpl_sb, xt8h)]
                nmm = len(terms) * (KO // 2)
                i = 0
                for wsb, xsb in terms:
                    for j in range(KO // 2):
                        nc.tensor.matmul(
                            psp[:], wsb[:, 2 * j:2 * j + 2, h * HD:(h + 1) * HD],
                            xsb[:, 2 * j:2 * j + 2, :],
                            start=(i == 0), stop=(i == nmm - 1), perf_mode=DR)
                        i += 1
                # phi = psp/64 + bphi
                nc.scalar.activation(phiT[:, h, sl], psp[:], IDENT,
                                     bias=bphi_sb[:, h:h + 1], scale=inv64)

        # Wo loads overlay the fp8 weight slots (ring waits for last readers)
        Wo_sb = {}
        for nm, dr in (("a", Wo8a), ("r", Wo8r), ("b", Wo8b)):
            wo_h = w8p.tile([P, NHL, HID], F8, tag="w8", name=f"wo{nm}")
            nc.scalar.dma_start(wo_h[:], dr)
            Wo_sb[nm] = wo_h

        # ================= phase B =================
        # Qg (fp32) in [d, h] layout (partitions = d)
        qg_f = stb.tile([P, NHL], F32, tag="qg")
        for h in range(NHL):
            nc.vector.tensor_reduce(
                qg_f[:, h:h + 1], qg_parts[:, h, :], AX, ADD)
        nc.vector.tensor_scalar_mul(qg_f[:], qg_f[:], 1.0 / S)

        # logits[s,h] = sum_d Kk[s,d] qg[d,h]: matmul from resident KkT into psum
        plog = pq.tile([P, NST, NHL], F32, tag="q")   # persists through softmax
        for st in range(NST):
            nc.tensor.matmul(plog[:, st, :], KkT[:, st * P:(st + 1) * P],
                             qg_f[:], start=True, stop=True)

        # softmax (exact, fp32), all 4 heads batched -> alpha_sd
        pmax1 = stb.tile([P, 1], F32, tag="pm")
        nc.vector.tensor_reduce(pmax1[:], plog.rearrange("p t h -> p (t h)"), AX, MAX)
        pmt1 = pr.tile([1, P], F32, tag="r")
        nc.tensor.transpose(pmt1[:], pmax1[:], ident_f32[:])
        gmt = stb.tile([1, P], F32, tag="gm")
        nc.vector.tensor_copy(gmt[:], pmt1[:])
        gmax1 = stb.tile([1, 1], F32, tag="g4")
        nc.vector.tensor_reduce(gmax1[:], gmt[:], AX, MAX)
        pngm = pkv.tile([P, 1], F32, tag="kv")
        nc.tensor.matmul(pngm[:], negr_f32[:], gmax1[:], start=True, stop=True)
        ngm1 = stb.tile([P, 1], F32, tag="ng")
        nc.vector.tensor_copy(ngm1[:], pngm[:])
        srow4 = stb.tile([P, NHL], F32, tag="sr")
        for h in range(NHL):
            nc.scalar.activation(alpha_sd[:, h, :], plog.rearrange("p t h -> p h t")[:, h, :], EXP,
                                 bias=ngm1[:], accum_out=srow4[:, h:h + 1])
        ptot4 = pr.tile([NHL, 1], F32, tag="r")
        nc.tensor.matmul(ptot4[:], srow4[:], ones_f32[:], start=True, stop=True)
        rcp4 = stb.tile([NHL, 1], F32, tag="r4")
        nc.vector.reciprocal(rcp4[:], ptot4[:])
        prr = pr.tile([1, NHL], F32, tag="r")
        nc.tensor.transpose(prr[:], rcp4[:], ident_f32[:NHL, :NHL])
        rcr = stb.tile([1, NHL], F32, tag="rr")
        nc.vector.tensor_copy(rcr[:], prr[:])
        prc4 = pkv.tile([P, NHL], F32, tag="kv")
        nc.tensor.matmul(prc4[:], srow_S[:], rcr[:], start=True, stop=True)
        rcpb4 = stb.tile([P, NHL], F32, tag="rb")
        nc.vector.tensor_copy(rcpb4[:], prc4[:])
        # alpha_sd now holds e = exp(l - gmax); normalization (S/total) is folded
        # into the outer_bf copy below.

        # outer[h] = sum_st (alpha*Kk)^T @ v
        COPY_F = mybir.ActivationFunctionType.Copy
        for h in range(NHL):
            pso = pkv.tile([P, HD], F32, tag="kv")
            for st in range(NST):
                kka = stk.tile([P, HD], BF16, tag="kka", bufs=4)
                if st % 4 != 3:
                    nc.vector.tensor_scalar_mul(
                        kka[:], Kk32[:, st, :], alpha_sd[:, h, st:st + 1])
                else:
                    nc.scalar.activation(kka[:], Kk32[:, st, :], COPY_F,
                                         scale=alpha_sd[:, h, st:st + 1])
                nc.tensor.matmul(pso[:], kka[:], v_sd[:, st, :],
                                 start=(st == 0), stop=(st == NST - 1))
            nc.scalar.activation(outer_bf[:, h, :], pso[:], COPY_F,
                                 scale=rcpb4[:, h:h + 1])

        # ============ phase C (pipelined; ctx quantized to fp8 hi/lo pair) ============
        # ctx = phiT*result; out = 1024*(hi8@Wo8a + hi8@Wo8r + lo8@Wo8b), DoubleRow
        def ctx_tiles(c):
            ctx_h8 = stc.tile([P, NHL, CS], F8, tag="cx8h", bufs=3, name=f"ctxh{c}")
            ctx_l8 = stc.tile([P, NHL, CS], F8, tag="cx8l", bufs=2, name=f"ctxl{c}")
            return ctx_h8, ctx_l8

        def resctx_head(c, h, tiles):
            sl = slice(c * CS, (c + 1) * CS)
            ctx_h8, ctx_l8 = tiles
            ppool, ptag = [(pq, "q"), (pph, "ph"), (pkv, "kv")][h % 3]
            psr2 = ppool.tile([P, CS], F32, tag=ptag, name=f"psr2_{c}_{h}")
            nc.tensor.matmul(psr2[:], outer_bf[:, h, :], QkT[:, h, sl],
                             start=True, stop=True)
            ctx32 = stc.tile([P, CS], F32, tag="c32", bufs=2, name=f"c32_{c}_{h}")
            nc.vector.scalar_tensor_tensor(
                ctx32[:], psr2[:], 2.0 ** -12, phiT[:, h, sl], MULT, MULT)
            nc.vector.tensor_scalar_mul(ctx_h8[:, h, :], ctx32[:], 2.0 ** -4)
            nc.vector.scalar_tensor_tensor(
                ctx_l8[:, h, :], ctx_h8[:, h, :], -16.0, ctx32[:], MULT, ADD)

        def ogroup(c, ch8, cl8, st, n, on_dve):
            # n in {0,1}: two 512-col psum groups merged into one 2KB-wide out DMA
            stg = c * (CS // P) + st
            ss = slice(st * P, (st + 1) * P)
            ob = sto.tile([P, 1024], BF16, tag="ob", bufs=2, name=f"ob{c}_{st}_{n}")
            for half in range(2):
                nn = n * 2 + half
                po_pool, po_tag = [(pq, "q"), (pph, "ph"), (pkv, "kv")][(st * 4 + nn) % 3]
                po = po_pool.tile([P, 512], F32, tag=po_tag, name=f"po{c}_{st}_{nn}")
                ns = slice(nn * 512, (nn + 1) * 512)
                i = 0
                for csb, wsb in ((ch8, "a"), (ch8, "r"), (cl8, "b")):
                    for j in range(NHL // 2):
                        nc.tensor.matmul(
                            po[:], csb[:, 2 * j:2 * j + 2, ss],
                            Wo_sb[wsb][:, 2 * j:2 * j + 2, ns],
                            start=(i == 0), stop=(i == 5), perf_mode=DR)
                        i += 1
                hsl = slice(half * 512, (half + 1) * 512)
                if on_dve:
                    nc.vector.tensor_scalar_mul(ob[:, hsl], po[:], 1024.0)
                else:
                    nc.scalar.activation(ob[:, hsl], po[:], COPY_F, scale=1024.0)
            dq = nc.sync if n == 0 else nc.scalar
            dq.dma_start(out_r[:, stg, n * 1024:(n + 1) * 1024], ob[:])

        ctx_cur = ctx_tiles(0)
        for h in range(NHL):
            resctx_head(0, h, ctx_cur)
        for c in range(NCH):
            ctx_next = ctx_tiles(c + 1) if c + 1 < NCH else None
            ch8, cl8 = ctx_cur
            groups = [(st, n) for st in range(CS // P) for n in range(2)]
            for gi, (st, n) in enumerate(groups):
                if gi < NHL and ctx_next is not None:
                    resctx_head(c + 1, gi, ctx_next)
                ogroup(c, ch8, cl8, st, n, on_dve=(gi >= 3))
            ctx_cur = ctx_next

    nc.compile()
    return nc


def _host_prep(hidden_states, position_ids, Wq, Wk, Wv, Wo, Wphi, bphi):
    B = hidden_states.shape[0]
    inv_freq = (1.0 / (ROPE_THETA ** (np.arange(0, HD, 2, dtype=np.float32) / HD))).astype(np.float32)
    Rm = np.zeros((P, P), dtype=np.float32)
    Rm[np.arange(64), np.arange(64) + 64] = -1.0
    Rm[np.arange(64) + 64, np.arange(64)] = 1.0
    RT_np = np.ascontiguousarray(Rm.T).astype(BF)

    def pair8(w):
        hi = w.astype(E4)
        lo = (w - hi.astype(np.float32)).astype(E4)
        return hi, lo

    def wmaj(w):
        # [HID, M] -> [P, KO, M] partition-major
        return np.ascontiguousarray(w.reshape(KO, P, -1).transpose(1, 0, 2))

    in_maps = []
    for b in range(B):
        freqs = position_ids[b].astype(np.float32)[:, None] * inv_freq[None, :]
        emb = np.concatenate([freqs, freqs], axis=1)          # [S, 128]
        cos_b = np.cos(emb).astype(np.float32)
        sin_b = np.sin(emb).astype(np.float32)
        xT = np.ascontiguousarray(hidden_states[b].T).astype(np.float32)
        # pre-chunk: [HID, S] -> [NCH, P, KO, CS] (partition-contiguous DMA)
        def chunkx(a):
            return np.ascontiguousarray(
                a.reshape(KO, P, NCH, CS).transpose(2, 1, 0, 3))
        x8hf, x8lf = pair8(xT)
        x8h = chunkx(x8hf)
        x8l = chunkx(x8lf)
        x16 = chunkx(xT.astype(BF))
        cosT_b = np.ascontiguousarray(cos_b.T).astype(BF)
        sinT_b = np.ascontiguousarray(sin_b.T).astype(BF)
        # k tables pre-chunked: [S, HD] -> [NCH, P, 2, HD]
        def chunks(a):
            return np.ascontiguousarray(
                a.reshape(NCH, 2, P, HD).transpose(0, 2, 1, 3))
        cos_sdc = chunks(cos_b)
        sin_sdc = chunks(sin_b)
        for g in range(4):
            sl4 = slice(g * 512, (g + 1) * 512)
            sl1 = slice(g * 128, (g + 1) * 128)
            wph, wpl = pair8(np.ascontiguousarray(Wphi[:, sl4]) * WSCALE)
            wo64 = np.ascontiguousarray(
                Wo[sl4, :].reshape(NHL, P, HID).transpose(1, 0, 2)).astype(np.float32) * WSCALE
            wo8a = wo64.astype(E4)
            wo8r = (wo64 - wo8a.astype(np.float32)).astype(E4)
            wo8b = (wo8a.astype(np.float32) / 16.0).astype(E4)
            in_maps.append({
                "x16": x16, "x8h": x8h, "x8l": x8l,
                "cosT": cosT_b, "sinT": sinT_b,
                "cos_sd": cos_sdc, "sin_sd": sin_sdc,
                "Wq8": wmaj((np.ascontiguousarray(Wq[:, sl4]) * WSCALE).astype(E4)),
                "Wkv": wmaj(np.ascontiguousarray(
                    np.concatenate([Wk[:, sl1], Wv[:, sl1]], axis=1)).astype(BF)),
                "Wph": wmaj(wph), "Wpl": wmaj(wpl),
                "Wo8a": wo8a, "Wo8b": wo8b, "Wo8r": wo8r,
                "bphi": np.ascontiguousarray(bphi[sl4]).astype(np.float32),
                "RT": RT_np,
            })
    return in_maps


def kernel(hidden_states, position_ids, Wq, Wk, Wv, Wo, Wphi, bphi, _trace=False):
    if "nc" not in _CACHE:
        _CACHE["nc"] = _build()
    nc = _CACHE["nc"]
    in_maps = _host_prep(np.asarray(hidden_states), np.asarray(position_ids),
                         np.asarray(Wq), np.asarray(Wk), np.asarray(Wv),
                         np.asarray(Wo), np.asarray(Wphi), np.asarray(bphi))
    res = run_bass_kernel_spmd(nc, in_maps, list(range(8)), trace=_trace)
    _CACHE["last_res"] = res
    B = hidden_states.shape[0]
    out = np.empty((B, S, HID), dtype=np.float32)
    for b in range(B):
        acc = res.results[b * 4 + 0]["out"].astype(np.float32)
        for g in range(1, 4):
            acc = acc + res.results[b * 4 + g]["out"].astype(np.float32)
        out[b] = acc
    return out
